# revision 20
# baseline (speedup 1.0000x reference)
"""GQA kernel for Trainium2 (Bass/Tile), 8-core head-parallel. v2.

Problem: x(1,2048,1024), Wq(1024,1024)+bq, Wk/Wv(1024,256)+bk/bv,
16 Q heads / 4 KV heads, head_dim 64, full (non-causal) softmax attention.
Reference output is attn(B,H,S,Dh) reshaped DIRECTLY to (B,S,H*Dh):
out rows [h*128,(h+1)*128) of the (2048,1024) output belong to head h.

Sharding: core d owns Q heads {2d, 2d+1} (both share KV head d//2), so each
core computes a contiguous (256,1024) slab of the final output.

Host-side prep (free): x transposed+cast to bf16 xT (1024,2048); per-core
weight slices pre-scaled (Wq/8 folds 1/sqrt(64)) and packed Wkv=[Wk|Wv],
all cast to bf16.

v2 structure (vs v1: 148387ns -> target ~75000ns):
  - xT DMAs batched 4-chunks-per-descriptor, split sync/gpsimd queues,
    block-major so block 0 lands first.
  - PE warmup matmuls at t0 (HAM warms during the DMA ramp).
  - B(bb) projects block bb; attention for q-block 0 is wave-interleaved
    with B so the scalar engine starts exp'ing at ~5us, not after B.
  - Score MMs for h0/h1 are adjacent with disjoint row groups
    (tile_position (0,0)/(64,0)) -> they run concurrently on the PE.
  - exp split: most k-blocks on Scalar (exact exp -> fp16), a few on the
    DVE as Schraudolph fastexp (x*1477.32+magic -> int16, bitcast fp16).
    All exps compute exp(s-2) (cancels in softmax; keeps fp16/fp8 range).
  - scores(kb+1) emitted before PV(kb) so the PE never idles on exp.
  - PSUM budget exactly 8 banks: scores 2x[128,1024] (4) + pso h0/h1 (2)
    + proj/transpose ring (2).
"""

import numpy as np

import concourse.bass as bass
import concourse.mybir as mybir
import concourse.tile as tile
from concourse import bacc
from concourse.bass_utils import run_bass_kernel_spmd
from concourse.masks import make_identity

F32 = mybir.dt.float32
BF16 = mybir.dt.bfloat16
F16 = mybir.dt.float16
I16 = mybir.dt.int16
AF = mybir.ActivationFunctionType
ALU = mybir.AluOpType

S = 2048
DIM = 1024
HD = 64
N_CORES = 8
NCH = DIM // 128   # 8 contraction chunks

SHIFT = -2.0                      # exp(s+SHIFT), cancels in softmax
K_FE = 1024 * 1.4426950408889634  # fp16 fastexp slope
MAGIC = 15360.0 - 29.0 + SHIFT * 1.4426950408889634 * 1024  # +29: sawtooth centering

# k-blocks handled by DVE fastexp instead of Scalar exp. During q-block 0
# the pipeline is PE-paced (projections share the PE), so the scalar engine
# has slack -> all-exact there; q1-3 are exp-paced -> offload 4/16 to DVE.
DVE_KBS = (2, 6, 10, 14)


def dve_kb(qb, kb):
    return qb > 0 and kb in DVE_KBS


def build_kernel():
    nc = bacc.Bacc("TRN2", target_bir_lowering=False, debug=False, num_devices=N_CORES)

    # weights host-prearranged to [128, chunk, 128] so the DMA is contiguous
    xt_d = nc.dram_tensor("xt", [DIM, S], BF16, kind="ExternalInput").ap()
    wq_d = nc.dram_tensor("wq", [128, NCH, 128], BF16, kind="ExternalInput").ap()
    wkv_d = nc.dram_tensor("wkv", [128, NCH, 128], BF16, kind="ExternalInput").ap()
    b_d = nc.dram_tensor("b", [128, 2], F32, kind="ExternalInput").ap()
    o_d = nc.dram_tensor("o", [2, S, HD], F32, kind="ExternalOutput").ap()

    with tile.TileContext(nc) as tc:
        with (
            tc.tile_pool(name="const", bufs=1) as const_pool,
            tc.tile_pool(name="persist", bufs=1) as persist_pool,
            tc.tile_pool(name="pt", bufs=3) as pt_pool,
            tc.tile_pool(name="outs", bufs=2) as out_pool,
            tc.tile_pool(name="ps_s", bufs=2, space="PSUM") as ps_s,
            tc.tile_pool(name="ps_o", bufs=1, space="PSUM") as ps_o,
            tc.tile_pool(name="ps_m", bufs=2, space="PSUM") as ps_m,
        ):
            # ---- constants (ident on gpsimd FIRST so warmup can start asap) ----
            ident = const_pool.tile([128, 128], F32)
            make_identity(nc, ident[:])
            ident2 = const_pool.tile([128, 64], BF16)
            nc.vector.tensor_copy(ident2[0:64, :], ident[0:64, 0:64])
            nc.sync.dma_start(ident2[64:128, :], ident2[0:64, :])

            # ---- persistent SBUF ----
            xT = persist_pool.tile([128, NCH, S], BF16)    # 4 MB
            qt_sb = persist_pool.tile([128, S], BF16)      # rows h*64+d
            kv_sb = persist_pool.tile([128, S], BF16)      # 0:64 KT, 64:128 VT
            kt2u = persist_pool.tile([128, S], BF16)       # KT dup at rows 64:128
            v_sb = persist_pool.tile([128, 16, 65], BF16)  # V' chunks + ones col

            # ---- PE warmup (HAM warms during the DMA ramp; garbage data) ----
            for w in range(2):
                warm = ps_m.tile([64, 256], F32, tag="proj")
                for r in range(8):
                    nc.tensor.matmul(warm[:], ident2[0:64, :], qt_sb[0:64, 0:256],
                                     start=(r == 0), stop=(r == 7),
                                     skip_group_check=True)

            # ---- input DMAs ----
            # measured queue rates: gpsimd(swdge) ~208 GB/s, sync ~124, scalar
            # ~64. Critical path to the first score MM: weights + xT block 0.
            wq_sb = const_pool.tile([128, NCH, 128], BF16)
            wkv_sb = const_pool.tile([128, NCH, 128], BF16)
            b_sb = const_pool.tile([128, 2], F32)
            nc.gpsimd.dma_start(wkv_sb[:], wkv_d[:])
            # bias is 128 tiny packets (~2.4us) -- scalar queue, never sync
            nc.scalar.dma_start(b_sb[:], b_d[:])
            bq_sb = b_sb[:, 0:1]
            bkv_sb = b_sb[:, 1:2]

            xt4 = xt_d.rearrange("(g p) s -> p g s", p=128)  # g: 8 chunks
            # block 0 per-chunk so projection MMs start with the first chunk
            # and the PE ramps (HAM) while the rest streams in.
            s0 = slice(0, 512)
            for c in range(5):
                nc.sync.dma_start(xT[:, c:c + 1, s0], xt4[:, c:c + 1, s0])
            for c in range(5, 8):
                nc.gpsimd.dma_start(xT[:, c:c + 1, s0], xt4[:, c:c + 1, s0])
            # emission (=accumulation) order for block-0 projections, by
            # expected DMA arrival across the two queues
            B0_ORDER = (0, 5, 1, 6, 7, 2, 3, 4)
            nc.gpsimd.dma_start(wq_sb[:], wq_d[:])
            s1 = slice(512, 1024)
            nc.scalar.dma_start(xT[:, 0:3, s1], xt4[:, 0:3, s1])
            nc.sync.dma_start(xT[:, 3:6, s1], xt4[:, 3:6, s1])
            nc.gpsimd.dma_start(xT[:, 6:8, s1], xt4[:, 6:8, s1])
            s23 = slice(1024, 2048)  # 2KB contiguous segments per partition
            qs = [nc.scalar, nc.sync, nc.gpsimd, nc.gpsimd]
            for j in range(4):
                qs[j].dma_start(xT[:, 2 * j:2 * j + 2, s23], xt4[:, 2 * j:2 * j + 2, s23])

            # small consts on vector (gpsimd queue stays clear for xT)
            for kb in range(16):
                nc.vector.memset(v_sb[:, kb, 64:65], 1.0)
            shift_sb = const_pool.tile([128, 1], F32)
            nc.vector.memset(shift_sb[:], SHIFT)

            # ---- helpers ----
            def proj_kv(bb, order=tuple(range(NCH))):
                # kv first: the kv->bias->kt2u-dup chain gates the h1 scores
                sl = slice(bb * 512, (bb + 1) * 512)
                pskv = ps_m.tile([128, 512], F32, tag="proj")
                for i, c in enumerate(order):
                    nc.tensor.matmul(pskv[:], wkv_sb[:, c, :], xT[:, c, sl],
                                     start=(i == 0), stop=(i == NCH - 1))
                nc.vector.tensor_scalar_add(kv_sb[:, sl], pskv[:], bkv_sb[:])
                nc.sync.dma_start(kt2u[64:128, sl], kv_sb[0:64, sl])

            def proj_q(bb, order=tuple(range(NCH))):
                sl = slice(bb * 512, (bb + 1) * 512)
                psq = ps_m.tile([128, 512], F32, tag="proj")
                for i, c in enumerate(order):
                    nc.tensor.matmul(psq[:], wq_sb[:, c, :], xT[:, c, sl],
                                     start=(i == 0), stop=(i == NCH - 1))
                nc.vector.tensor_scalar_add(qt_sb[:, sl], psq[:], bq_sb[:])

            def vtr(bb):
                for j in range(4):
                    kb = bb * 4 + j
                    ps = ps_m.tile([128, 64], BF16, tag="proj")
                    nc.tensor.matmul(
                        ps[:], kv_sb[64:128, kb * 128:(kb + 1) * 128],
                        ident2[64:128, :], is_transpose=True)
                    nc.vector.tensor_copy(v_sb[:, kb, 0:64], ps[:])

            def emit_scores(qsl, kb):
                """score pair for (h0,h1) at k-block kb -> [128,1024] psum."""
                pss = ps_s.tile([128, 1024], F32, tag="s")
                kcols = slice(kb * 128, (kb + 1) * 128)
                nc.tensor.matmul(pss[:, 0:512], kv_sb[0:64, kcols],
                                 qt_sb[0:64, qsl], start=True, stop=True)
                nc.tensor.matmul(pss[:, 512:1024], kt2u[64:128, kcols],
                                 qt_sb[64:128, qsl], start=True, stop=True)
                return pss

            def emit_exp(pss, qb, kb):
                pt = pt_pool.tile([128, 1024], F16)
                if dve_kb(qb, kb):
                    nc.vector.tensor_scalar(
                        pt[:].bitcast(I16), pss[:], K_FE, MAGIC, ALU.mult, ALU.add)
                else:
                    nc.scalar.activation(pt[:], pss[:], AF.Exp, bias=shift_sb[:])
                return pt

            def emit_pv(pso_h0, pso_h1, pt, kb):
                nc.tensor.matmul(pso_h0[:], v_sb[:, kb, :], pt[:, 0:512],
                                 start=(kb == 0), stop=(kb == 15),
                                 skip_group_check=True)
                nc.tensor.matmul(pso_h1[:], v_sb[:, kb, :], pt[:, 512:1024],
                                 start=(kb == 0), stop=(kb == 15),
                                 skip_group_check=True)

            def emit_output(qb, pso_h0, pso_h1):
                qsl = slice(qb * 512, (qb + 1) * 512)
                ot_sb = out_pool.tile([65, 2, 512], F32, tag="ot_sb")
                nc.vector.tensor_copy(ot_sb[:, 0, :], pso_h0[:])
                nc.vector.tensor_copy(ot_sb[:, 1, :], pso_h1[:])
                o_sbs = []
                for h in range(2):
                    o_sb = out_pool.tile([128, 4, HD], F32, tag=f"o_sb{h}")
                    o_sbs.append(o_sb)
                    for j in range(4):
                        ps = ps_m.tile([128, 65], F32, tag="proj")
                        nc.tensor.transpose(
                            ps[:], ot_sb[:, h, j * 128:(j + 1) * 128],
                            ident[:65, :65])
                        rcp = out_pool.tile([128, 1], F32, tag="rcp")
                        nc.vector.reciprocal(rcp[:], ps[:, 64:65])
                        nc.vector.tensor_scalar_mul(o_sb[:, j, :], ps[:, 0:64], rcp[:])
                for h in range(2):
                    nc.sync.dma_start(
                        o_d[h, qsl, :].rearrange("(t j) c -> j t c", j=128),
                        o_sbs[h][:])

            # ---- fused B + C(q0) wave pipeline ----
            # B(0) first, then per wave (4 kb of q0 attention) interleave the
            # next block's projection pieces so scores trickle out evenly and
            # the scalar engine never starves.
            q0 = slice(0, 512)
            pso_h0 = ps_o.tile([65, 512], F32, tag="oh0")
            pso_h1 = ps_o.tile([65, 512], F32, tag="oh1")
            proj_kv(0, B0_ORDER)
            proj_q(0, B0_ORDER)
            vtr(0)
            pend = []  # (pt, kb) waiting for PV emit (scores stay 1 ahead)
            for bb in range(4):
                for kb in range(bb * 4, bb * 4 + 4):
                    pss = emit_scores(q0, kb)
                    pt = emit_exp(pss, 0, kb)
                    pend.append((pt, kb))
                    if len(pend) > 1:
                        p, k = pend.pop(0)
                        emit_pv(pso_h0, pso_h1, p, k)
                    if bb < 3:
                        j = kb % 4
                        if j == 0:
                            proj_kv(bb + 1)
                        elif j == 1:
                            proj_q(bb + 1)
                        elif j == 2:
                            vtr(bb + 1)
            p, k = pend.pop(0)
            emit_pv(pso_h0, pso_h1, p, k)
            emit_output(0, pso_h0, pso_h1)

            # ---- C(q1..q3) ----
            for qb in range(1, 4):
                qsl = slice(qb * 512, (qb + 1) * 512)
                pso_h0 = ps_o.tile([65, 512], F32, tag="oh0")
                pso_h1 = ps_o.tile([65, 512], F32, tag="oh1")
                pend = []
                for kb in range(16):
                    pss = emit_scores(qsl, kb)
                    pt = emit_exp(pss, qb, kb)
                    pend.append((pt, kb))
                    if len(pend) > 1:
                        p, k = pend.pop(0)
                        emit_pv(pso_h0, pso_h1, p, k)
                p, k = pend.pop(0)
                emit_pv(pso_h0, pso_h1, p, k)
                emit_output(qb, pso_h0, pso_h1)

    nc.compile()
    return nc


_NC_CACHE = None


def make_in_maps(inputs):
    import ml_dtypes
    x = np.asarray(inputs["x"], np.float32).reshape(S, DIM)
    xt = np.ascontiguousarray(x.T).astype(ml_dtypes.bfloat16)
    Wq = np.asarray(inputs["Wq"], np.float32)
    bq = np.asarray(inputs["bq"], np.float32)
    Wk = np.asarray(inputs["Wk"], np.float32)
    bk = np.asarray(inputs["bk"], np.float32)
    Wv = np.asarray(inputs["Wv"], np.float32)
    bv = np.asarray(inputs["bv"], np.float32)

    in_maps = []
    for d in range(N_CORES):
        g = d // 2
        wkv = np.concatenate(
            [Wk[:, g * 64:(g + 1) * 64], Wv[:, g * 64:(g + 1) * 64]], axis=1)
        bkv = np.concatenate([bk[g * 64:(g + 1) * 64], bv[g * 64:(g + 1) * 64]])
        wq_s = (Wq[:, d * 128:(d + 1) * 128] / 8.0).astype(ml_dtypes.bfloat16)
        wkv_s = wkv.astype(ml_dtypes.bfloat16)
        b2 = np.stack([bq[d * 128:(d + 1) * 128] / 8.0, bkv], axis=1)
        in_maps.append({
            "xt": xt,
            # [1024,128] -> [128 partition, 8 chunk, 128] contiguous
            "wq": np.ascontiguousarray(wq_s.reshape(NCH, 128, 128).transpose(1, 0, 2)),
            "wkv": np.ascontiguousarray(wkv_s.reshape(NCH, 128, 128).transpose(1, 0, 2)),
            "b": np.ascontiguousarray(b2, dtype=np.float32),
        })
    return in_maps


def kernel(**inputs) -> np.ndarray:
    global _NC_CACHE
    if _NC_CACHE is None:
        _NC_CACHE = build_kernel()
    nc = _NC_CACHE
    in_maps = make_in_maps(inputs)
    res = run_bass_kernel_spmd(nc, in_maps, list(range(N_CORES)))
    blocks = [np.asarray(res.results[d]["o"]).reshape(256, DIM) for d in range(N_CORES)]
    return np.concatenate(blocks, axis=0).reshape(1, S, DIM).astype(np.float32)
